# revision 5
# baseline (speedup 1.0000x reference)
"""AugmentedTripletLoss on 8 TRN2 NeuronCores — data-parallel Bass kernel.

Strategy (data-parallel over batch, 16384 samples/core):
  Phase A (single pass over HBM): per 128-sample tile, compute row norms,
    normalized embeddings (bf16, kept resident in SBUF transposed), one-hot
    label masks, and accumulate class sums (via norm-scaled one-hot matmul)
    and class counts in PSUM.
  AllReduce #1: [16, 513] (class embedding sums ++ counts).
  Phase B (tiny): centroids, normalized centroids, pair mask pm, deg,
    pmsym = pm + pm^T.
  Phase C (SBUF-resident, no HBM): dot = ehatT.T @ chatT per tile, then
    relu terms, accumulate S^T[16,16] and per-class intra sums t[16,1] in
    PSUM via one-hot matmuls.
  AllReduce #2: [16, 17] (S^T ++ t). Final scalar assembled on-device.
"""

import sys

sys.path.insert(0, "/opt/trn_rl_repo")

import numpy as np

import concourse.bass as bass
import concourse.bacc as bacc
import concourse.tile as tile
import concourse.mybir as mybir
from concourse.bass_utils import run_bass_kernel_spmd

ALPHA = 0.1
BETA = 1.1
C = 16
N = 131072
D = 512
CORES = 8
NL = N // CORES  # 16384 samples per core
P = 128
T = NL // P  # 128 tiles per core
KCH = D // P  # 4 contraction chunks of 128

F32 = mybir.dt.float32
BF16 = mybir.dt.bfloat16
ALU = mybir.AluOpType
ACTF = mybir.ActivationFunctionType

_CACHE = {}


def _build():
    nc = bacc.Bacc("TRN2", target_bir_lowering=False, debug=False, num_devices=CORES)

    emb = nc.dram_tensor("emb", [NL, D], F32, kind="ExternalInput")
    lab = nc.dram_tensor("lab", [P, T], F32, kind="ExternalInput")
    out = nc.dram_tensor("out", [1, 1], F32, kind="ExternalOutput")
    rg = [list(range(CORES))]

    with tile.TileContext(nc) as tc:
        with (
            tc.tile_pool(name="pers", bufs=1) as pers,
            tc.tile_pool(name="work", bufs=3) as work,
            tc.tile_pool(name="small", bufs=1) as small,
            tc.tile_pool(name="psacc", bufs=1, space="PSUM") as psacc,
            tc.tile_pool(name="pstr", bufs=2, space="PSUM") as pstr,
            tc.tile_pool(name="dram", bufs=1, space="DRAM") as dram,
        ):
            # ---- persistent SBUF state ----
            ehatT = pers.tile([P, KCH * NL], BF16)   # transposed normalized emb
            ohb = pers.tile([P, T * C], BF16)        # one-hot per tile (bf16)
            lab_sb = pers.tile([P, T], F32)
            iota_cls = pers.tile([P, C], F32)
            i128 = pers.tile([P, P], BF16)
            i16 = pers.tile([C, C], F32)
            ones_bf = pers.tile([P, 1], BF16)
            ones16 = pers.tile([C, 1], F32)
            chT = pers.tile([P, KCH * C], BF16)      # transposed normalized centroids

            # constants
            nc.sync.dma_start(lab_sb[:], lab[:, :])
            nc.gpsimd.iota(iota_cls[:], [[1, C]], channel_multiplier=0,
                           allow_small_or_imprecise_dtypes=True)
            iota_p128 = small.tile([P, 1], F32)
            nc.gpsimd.iota(iota_p128[:], [[0, 1]], channel_multiplier=1,
                           allow_small_or_imprecise_dtypes=True)
            iota_r128 = small.tile([P, P], F32)
            nc.gpsimd.iota(iota_r128[:], [[1, P]], channel_multiplier=0,
                           allow_small_or_imprecise_dtypes=True)
            nc.vector.tensor_scalar(i128[:], iota_r128[:], iota_p128[:], None,
                                    ALU.is_equal)
            nc.vector.tensor_scalar(i16[:], iota_cls[:C, :], iota_p128[:C, :], None,
                                    ALU.is_equal)
            nc.vector.memset(ones_bf[:], 1.0)
            nc.vector.memset(ones16[:], 1.0)
            zb = pers.tile([P, 1], F32)
            nc.vector.memset(zb[:], 0.0)
            bq = pers.tile([P, 1], F32)
            nc.vector.memset(bq[:], float(BETA - 1.0))
            br = pers.tile([P, 1], F32)
            nc.vector.memset(br[:], float(1.0 - ALPHA))

            ps_sums = psacc.tile([C, D], F32)
            ps_cnt = psacc.tile([C, 1], F32)

            # ================= Phase A =================
            for t in range(T):
                e = work.tile([P, D], F32)
                nc.sync.dma_start(e[:], emb[t * P:(t + 1) * P, :])

                sq = work.tile([P, D], BF16)
                ssq = work.tile([P, 1], F32)
                nc.scalar.activation(sq[:], e[:], ACTF.Square, bias=zb[:], accum_out=ssq[:])
                nrm = work.tile([P, 1], F32)
                nc.scalar.activation(nrm[:], ssq[:], ACTF.Sqrt, bias=zb[:])
                nc.vector.tensor_scalar_max(nrm[:], nrm[:], 1e-8)
                rnrm = work.tile([P, 1], F32)
                nc.vector.reciprocal(rnrm[:], nrm[:])

                ehat = work.tile([P, D], BF16)
                nc.vector.tensor_scalar(ehat[:], e[:], rnrm[:], None, ALU.mult)

                oh = work.tile([P, C], F32)
                nc.vector.tensor_scalar(oh[:], iota_cls[:], lab_sb[:, t:t + 1], None,
                                        ALU.is_equal)
                ohn = work.tile([P, C], BF16)
                nc.vector.tensor_scalar(ohn[:], oh[:], nrm[:], None, ALU.mult)
                nc.any.tensor_copy(ohb[:, t * C:(t + 1) * C], oh[:])

                # class sums: sum_{s in c} e_s = sum onehot*||e|| * ehat
                nc.tensor.matmul(ps_sums[:], ohn[:], ehat[:],
                                 start=(t == 0), stop=(t == T - 1))
                nc.tensor.matmul(ps_cnt[:], ohb[:, t * C:(t + 1) * C], ones_bf[:],
                                 start=(t == 0), stop=(t == T - 1))

                # transpose ehat into persistent [d, s] layout
                for k in range(KCH):
                    tp = pstr.tile([P, P], BF16, tag="tp")
                    nc.tensor.transpose(tp[:], ehat[:, k * P:(k + 1) * P], i128[:])
                    nc.any.tensor_copy(ehatT[:, k * NL + t * P: k * NL + (t + 1) * P],
                                       tp[:])

            # ================= AllReduce #1 =================
            loc1 = small.tile([C, D + 1], F32)
            nc.vector.tensor_copy(loc1[:, :D], ps_sums[:])
            nc.vector.tensor_copy(loc1[:, D:D + 1], ps_cnt[:])
            ar1_in = dram.tile([C, D + 1], F32)
            ar1_out = dram.tile([C, D + 1], F32, addr_space="Shared")
            nc.gpsimd.dma_start(ar1_in[:], loc1[:])
            nc.gpsimd.collective_compute(
                "AllReduce", ALU.add, replica_groups=rg,
                ins=[ar1_in.opt()], outs=[ar1_out.opt()])
            g1 = small.tile([C, D + 1], F32)
            nc.gpsimd.dma_start(g1[:], ar1_out[:])

            # ================= Phase B (tiny) =================
            cnt = small.tile([C, 1], F32)
            nc.vector.tensor_copy(cnt[:], g1[:, D:D + 1])
            cdenom = small.tile([C, 1], F32)
            nc.vector.tensor_scalar_max(cdenom[:], cnt[:], 1.0)
            rcnt = small.tile([C, 1], F32)
            nc.vector.reciprocal(rcnt[:], cdenom[:])
            cent = small.tile([C, D], F32)
            nc.vector.tensor_scalar(cent[:], g1[:, :D], rcnt[:], None, ALU.mult)

            csq = small.tile([C, D], BF16)
            cssq = small.tile([C, 1], F32)
            nc.scalar.activation(csq[:], cent[:], ACTF.Square, bias=zb[:C, :], accum_out=cssq[:])
            cnrm = small.tile([C, 1], F32)
            nc.scalar.activation(cnrm[:], cssq[:], ACTF.Sqrt, bias=zb[:C, :])
            nc.vector.tensor_scalar_max(cnrm[:], cnrm[:], 1e-8)
            rcnrm = small.tile([C, 1], F32)
            nc.vector.reciprocal(rcnrm[:], cnrm[:])
            chat = small.tile([C, D], F32)
            nc.vector.tensor_scalar(chat[:], cent[:], rcnrm[:], None, ALU.mult)

            # chatT [d, c] via PE transpose, kept in bf16
            for k in range(KCH):
                tpc = pstr.tile([P, C], F32, tag="tiny")
                nc.tensor.transpose(tpc[:], chat[:, k * P:(k + 1) * P], i16[:])
                nc.any.tensor_copy(chT[:, k * C:(k + 1) * C], tpc[:])

            # pairwise centroid dots -> pm
            ps_pd = pstr.tile([C, C], F32, tag="tiny")
            for k in range(KCH):
                nc.tensor.matmul(ps_pd[:], chT[:, k * C:(k + 1) * C],
                                 chT[:, k * C:(k + 1) * C],
                                 start=(k == 0), stop=(k == KCH - 1))
            # condition: dot >= 1-BETA  (i.e. 1-dot <= BETA)
            cond = small.tile([C, C], F32)
            nc.vector.tensor_scalar(cond[:], ps_pd[:], float(1.0 - BETA), None,
                                    ALU.is_ge)
            upper = small.tile([C, C], F32)
            nc.vector.tensor_scalar(upper[:], iota_cls[:C, :], iota_p128[:C, :], None,
                                    ALU.is_gt)
            present = small.tile([C, 1], F32)
            nc.vector.tensor_scalar(present[:], cnt[:], 0.5, None, ALU.is_gt)
            # broadcast present over rows: ones16[1,16-as-K] trick
            presT = pstr.tile([1, C], F32, tag="tiny")
            nc.tensor.transpose(presT[:], present[:], i16[:])
            presT_sb = small.tile([1, C], F32)
            nc.vector.tensor_copy(presT_sb[:], presT[:])
            ones_r16 = small.tile([1, C], F32)
            nc.vector.memset(ones_r16[:], 1.0)
            presB = pstr.tile([C, C], F32, tag="tiny")
            nc.tensor.matmul(presB[:], ones_r16[:], presT_sb[:],
                             start=True, stop=True)

            pm = small.tile([C, C], F32)
            nc.vector.tensor_tensor(pm[:], cond[:], upper[:], ALU.mult)
            nc.vector.tensor_scalar(pm[:], pm[:], present[:], None, ALU.mult)
            nc.vector.tensor_tensor(pm[:], pm[:], presB[:], ALU.mult)

            deg = small.tile([C, 1], F32)
            nc.vector.tensor_reduce(deg[:], pm[:], mybir.AxisListType.X, ALU.add)
            ps_cs = pstr.tile([C, 1], F32, tag="tiny")
            nc.tensor.matmul(ps_cs[:], pm[:], ones16[:], start=True, stop=True)
            nc.vector.tensor_tensor(deg[:], deg[:], ps_cs[:], ALU.add)

            ps_pmT = pstr.tile([C, C], F32, tag="tiny")
            nc.tensor.transpose(ps_pmT[:], pm[:], i16[:])
            pmsym = small.tile([C, C], F32)
            nc.vector.tensor_tensor(pmsym[:], pm[:], ps_pmT[:], ALU.add)

            # ================= Phase C =================
            ps_st = psacc.tile([C, C + 1], F32)
            for t in range(T):
                dot = pstr.tile([P, C], F32, tag="tp")
                for k in range(KCH):
                    nc.tensor.matmul(dot[:], ehatT[:, k * NL + t * P: k * NL + (t + 1) * P],
                                     chT[:, k * C:(k + 1) * C],
                                     start=(k == 0), stop=(k == KCH - 1))
                qr = work.tile([P, C + 1], BF16)
                # inter term: relu(BETA - (1 - dot)) = relu(dot + (BETA-1))
                nc.scalar.activation(qr[:, :C], dot[:], ACTF.Relu,
                                     bias=bq[:], scale=1.0)
                # intra term: relu((1 - dot) - ALPHA) = relu(-dot + (1-ALPHA))
                rt = work.tile([P, C], F32)
                nc.scalar.activation(rt[:], dot[:], ACTF.Relu,
                                     bias=br[:], scale=-1.0)
                rr = work.tile([P, C], F32)
                rsum = work.tile([P, 1], F32)
                nc.vector.scalar_tensor_tensor(rr[:], rt[:], 1.0,
                                               ohb[:, t * C:(t + 1) * C],
                                               ALU.mult, ALU.mult,
                                               accum_out=rsum[:])
                nc.vector.tensor_copy(qr[:, C:C + 1], rsum[:])
                nc.tensor.matmul(ps_st[:], ohb[:, t * C:(t + 1) * C], qr[:],
                                 start=(t == 0), stop=(t == T - 1))

            # ================= AllReduce #2 =================
            loc2 = small.tile([C, C + 1], F32)
            nc.vector.tensor_copy(loc2[:], ps_st[:])
            ar2_in = dram.tile([C, C + 1], F32)
            ar2_out = dram.tile([C, C + 1], F32, addr_space="Shared")
            nc.gpsimd.dma_start(ar2_in[:], loc2[:])
            nc.gpsimd.collective_compute(
                "AllReduce", ALU.add, replica_groups=rg,
                ins=[ar2_in.opt()], outs=[ar2_out.opt()])
            g2 = small.tile([C, C + 1], F32)
            nc.gpsimd.dma_start(g2[:], ar2_out[:])

            # ================= final scalar =================
            # g2[:, :C] = S^T summed over cores; g2[:, C] = per-class intra sums t
            cat = small.tile([C, 4], F32)
            degt = small.tile([C, 1], F32)
            nc.vector.tensor_tensor(degt[:], deg[:], g2[:, C:C + 1], ALU.mult)
            nc.vector.tensor_copy(cat[:, 0:1], degt[:])
            inte = small.tile([C, C], F32)
            nc.vector.tensor_tensor(inte[:], pmsym[:], g2[:, :C], ALU.mult)
            nc.vector.tensor_reduce(cat[:, 1:2], inte[:], mybir.AxisListType.X,
                                    ALU.add)
            dcnt = small.tile([C, 1], F32)
            nc.vector.tensor_tensor(dcnt[:], deg[:], cnt[:], ALU.mult)
            nc.vector.tensor_copy(cat[:, 2:3], dcnt[:])
            nc.vector.tensor_reduce(cat[:, 3:4], pm[:], mybir.AxisListType.X,
                                    ALU.add)

            ps_fin = pstr.tile([4, 1], F32, tag="tiny")
            nc.tensor.matmul(ps_fin[:], cat[:], ones16[:], start=True, stop=True)
            fin = small.tile([4, 1], F32)
            nc.vector.tensor_copy(fin[:], ps_fin[:])
            ps_fr = pstr.tile([1, 4], F32, tag="tiny")
            nc.tensor.transpose(ps_fr[:], fin[:], i16[:4, :4])
            fr = small.tile([1, 4], F32)
            nc.vector.tensor_copy(fr[:], ps_fr[:])

            ia = small.tile([1, 1], F32)
            nc.vector.tensor_reduce(ia[:], fr[:, 0:2], mybir.AxisListType.X, ALU.add)
            den = small.tile([1, 1], F32)
            nc.vector.tensor_scalar_max(den[:], fr[:, 2:3], 1.0)
            rden = small.tile([1, 1], F32)
            nc.vector.reciprocal(rden[:], den[:])
            npos = small.tile([1, 1], F32)
            nc.vector.tensor_scalar(npos[:], fr[:, 3:4], 0.5, None, ALU.is_gt)
            loss = small.tile([1, 1], F32)
            nc.vector.tensor_tensor(loss[:], ia[:], rden[:], ALU.mult)
            nc.vector.tensor_tensor(loss[:], loss[:], npos[:], ALU.mult)
            nc.sync.dma_start(out.ap()[:, :], loss[:])

    nc.compile()
    return nc


def kernel(embeddings: np.ndarray, labels: np.ndarray) -> np.ndarray:
    emb = np.ascontiguousarray(np.asarray(embeddings, dtype=np.float32))
    labf = np.asarray(labels).astype(np.float32)

    if "nc" not in _CACHE:
        _CACHE["nc"] = _build()
    nc = _CACHE["nc"]

    in_maps = []
    for i in range(CORES):
        esh = emb[i * NL:(i + 1) * NL]
        lsh = np.ascontiguousarray(
            labf[i * NL:(i + 1) * NL].reshape(T, P).T)  # [P, T]
        in_maps.append({"emb": esh, "lab": lsh})

    res = run_bass_kernel_spmd(nc, in_maps, core_ids=list(range(CORES)))
    return np.float32(res.results[0]["out"].reshape(())[()])
